# revision 3
# baseline (speedup 1.0000x reference)
"""Trainium2 Bass kernel for nn_F_VAE_can_7902739824969.

Reference computes, per batch row b with domain d = dom[b]:
    out[b] = F_{d} @ eps[b] + concat(bias_shared, bias_nonshared[d])
where F_d = (I - L_d)^{-1} S_d, L_d strictly-lower only in the last K=64 rows,
S_d diagonal.  Hence F_d = [[I, 0], [F21_d, F22_d]] and only the bottom
K rows (F_bot = [F21|F22], shape [D, K, N]) carry information:
    out[b, :N-K] = eps[b, :N-K] + bias_shared
    out[b, N-K:] = F_bot[d] @ eps[b] + bias_nonshared[d]

Host side (inside kernel()): solve the D small unit-triangular systems for
F_bot, sort batch rows by domain (a sharding permutation), give each of the
8 cores 128 sorted rows plus only the few domain blocks of F^T that shard
touches.  Device: PE-transpose eps, one matmul group per domain segment with
the nonshared bias folded in via a ones-row, masked combine, shared-bias add,
DMA out.  Host inverse-permutes the gathered shards.
"""

import numpy as np

B = 1024
N = 512
K = 64
D = 16
P = 128
NC = 8
RPC = B // NC          # rows per core
NTOP = N - K           # 448
NCHUNK = N // P        # 4 contraction chunks
MAX_SEG_PER_BANK = 8   # K*8 fp32 = 2KB = one PSUM bank; also matmul N<=512

_PROG_CACHE: dict = {}


def _build_fbot(L_emb, S_emb):
    """F_bot [D, K, N] (float32): bottom K rows of (I - L_d)^{-1} S_d."""
    L_emb = np.asarray(L_emb, np.float64)
    S_emb = np.asarray(S_emb, np.float64)
    off = np.zeros(K, dtype=np.int64)
    for r in range(1, K):
        off[r] = off[r - 1] + (NTOP + r - 1)
    L21 = np.zeros((D, K, NTOP))
    L22 = np.zeros((D, K, K))
    for r in range(K):
        L21[1:, r, :] = L_emb[1:, off[r] : off[r] + NTOP]
        if r > 0:
            L22[1:, r, :r] = L_emb[1:, off[r] + NTOP : off[r] + NTOP + r]
    s = np.ones((D, K))
    s[1:] = S_emb[1:]
    rhs = np.concatenate([L21, s[:, :, None] * np.eye(K)[None]], axis=2)  # [D,K,N]
    X = np.zeros_like(rhs)
    for r in range(K):
        X[:, r, :] = rhs[:, r, :] + np.einsum(
            "dj,djn->dn", L22[:, r, :r], X[:, :r, :]
        )
    return X.astype(np.float32)


def _seg_layout(nseg):
    """Split nseg segments into PSUM banks of <= MAX_SEG_PER_BANK."""
    banks = []
    s0 = 0
    while s0 < nseg:
        nb = min(MAX_SEG_PER_BANK, nseg - s0)
        banks.append((s0, nb))
        s0 += nb
    return banks


def _build_program(nseg):
    import concourse.bacc as bacc
    import concourse.mybir as mybir
    import concourse.tile as tile
    from concourse.masks import make_identity

    banks = _seg_layout(nseg)
    fta_cols = K * nseg
    f32 = mybir.dt.float32

    nc = bacc.Bacc()
    eps_in = nc.declare_dram_parameter("eps", [RPC, N], f32, isOutput=False)
    fta_in = nc.declare_dram_parameter("fta", [N, fta_cols], f32, isOutput=False)
    fbias_in = nc.declare_dram_parameter("fbias", [1, fta_cols], f32, isOutput=False)
    masks_in = nc.declare_dram_parameter("masks", [RPC, nseg], f32, isOutput=False)
    bsh_in = nc.declare_dram_parameter("bsh", [1, NTOP], f32, isOutput=False)
    out_ext = nc.declare_dram_parameter("out", [RPC, N], f32, isOutput=True)

    with tile.TileContext(nc) as tc:
        with (
            tc.tile_pool(name="sbuf", bufs=1) as sb,
            tc.tile_pool(name="psum", bufs=1, space="PSUM") as ps,
        ):
            ident = sb.tile([P, P], f32, tag="ident")
            make_identity(nc, ident[:])
            ones = sb.tile([1, P], f32, tag="ones")
            nc.vector.memset(ones[:], 1.0)

            eps_sb = sb.tile([P, N], f32, tag="eps")
            nc.sync.dma_start(eps_sb[:], eps_in[:])
            masks_sb = sb.tile([P, nseg], f32, tag="masks")
            nc.sync.dma_start(masks_sb[:], masks_in[:])
            bsh_sb = sb.tile([1, NTOP], f32, tag="bsh")
            nc.sync.dma_start(bsh_sb[:], bsh_in[:])
            fbias_sb = sb.tile([1, fta_cols], f32, tag="fbias")
            nc.sync.dma_start(fbias_sb[:], fbias_in[:])
            fta_sb = []
            for c in range(NCHUNK):
                t = sb.tile([P, fta_cols], f32, tag=f"fta{c}")
                nc.sync.dma_start(t[:], fta_in[c * P : (c + 1) * P, :])
                fta_sb.append(t)

            # eps^T via PE transpose (n on partitions for the contraction)
            epsT = sb.tile([P, N], f32, tag="epsT")
            for c in range(NCHUNK):
                pt = ps.tile([P, P], f32, tag=f"tr{c}")
                nc.tensor.transpose(pt[:], eps_sb[:, c * P : (c + 1) * P], ident[:])
                nc.vector.tensor_copy(epsT[:, c * P : (c + 1) * P], pt[:])

            # shared-bias broadcast: ones^T @ bsh -> [P, NTOP]
            ptop = ps.tile([P, NTOP], f32, tag="ptop")
            nc.tensor.matmul(ptop[:], lhsT=ones[:], rhs=bsh_sb[:], start=True, stop=True)

            out_sb = sb.tile([P, N], f32, tag="out")
            nc.vector.tensor_tensor(
                out=out_sb[:, :NTOP],
                in0=eps_sb[:, :NTOP],
                in1=ptop[:],
                op=mybir.AluOpType.add,
            )

            # per-bank Z matmuls: pz[b, k, s] = sum_n eps[b,n] F_bot[d_s,k,n] + bias_ns[d_s,k]
            for bi, (s0, nb) in enumerate(banks):
                cols = slice(K * s0, K * (s0 + nb))
                pz = ps.tile([P, K, nb], f32, tag=f"pz{bi}")
                for c in range(NCHUNK):
                    nc.tensor.matmul(
                        pz[:],
                        lhsT=epsT[:, c * P : (c + 1) * P],
                        rhs=fta_sb[c][:, cols],
                        start=(c == 0),
                        stop=False,
                    )
                nc.tensor.matmul(
                    pz[:], lhsT=ones[:], rhs=fbias_sb[:, cols], start=False, stop=True
                )
                # masked select: out_bot += sum_s pz[:, :, s] * masks[:, s]
                tmp = sb.tile([P, K, nb], f32, tag=f"tmp{bi}")
                nc.vector.tensor_tensor(
                    out=tmp[:],
                    in0=pz[:],
                    in1=masks_sb[:, None, s0 : s0 + nb].to_broadcast([P, K, nb]),
                    op=mybir.AluOpType.mult,
                )
                if bi == 0:
                    nc.vector.tensor_reduce(
                        out=out_sb[:, NTOP:N],
                        in_=tmp[:],
                        axis=mybir.AxisListType.X,
                        op=mybir.AluOpType.add,
                    )
                else:
                    red = sb.tile([P, K], f32, tag=f"red{bi}")
                    nc.vector.tensor_reduce(
                        out=red[:],
                        in_=tmp[:],
                        axis=mybir.AxisListType.X,
                        op=mybir.AluOpType.add,
                    )
                    nc.vector.tensor_tensor(
                        out=out_sb[:, NTOP:N],
                        in0=out_sb[:, NTOP:N],
                        in1=red[:],
                        op=mybir.AluOpType.add,
                    )

            nc.sync.dma_start(out_ext[:], out_sb[:])

    nc.compile()
    return nc


def _prepare(epsilon, d, L_emb, S_emb, bias_nonshared, bias_shared):
    """Host-side sharding. Returns (nseg, in_maps, perm)."""
    eps = np.ascontiguousarray(np.asarray(epsilon, np.float32))
    dv = np.asarray(d).astype(np.int64).reshape(B)
    bias_ns = np.asarray(bias_nonshared, np.float32)
    bias_sh = np.asarray(bias_shared, np.float32)

    fbot = _build_fbot(L_emb, S_emb)           # [D, K, N]
    ft = np.ascontiguousarray(fbot.transpose(0, 2, 1))  # [D, N, K]

    perm = np.argsort(dv, kind="stable")
    ds_sorted = dv[perm]
    eps_sorted = eps[perm]

    shard_segs = []
    for c in range(NC):
        rows = ds_sorted[c * RPC : (c + 1) * RPC]
        segs = []
        for dd in rows:
            if not segs or segs[-1] != dd:
                segs.append(int(dd))
        shard_segs.append(segs)
    nseg = max(len(s) for s in shard_segs)

    banks = _seg_layout(nseg)
    fta_cols = K * nseg
    in_maps = []
    for c in range(NC):
        segs = shard_segs[c]
        rows = ds_sorted[c * RPC : (c + 1) * RPC]
        fta = np.zeros((N, fta_cols), np.float32)
        fbias = np.zeros((1, fta_cols), np.float32)
        masks = np.zeros((RPC, nseg), np.float32)
        for s0, nb in banks:
            for sl in range(nb):
                s = s0 + sl
                if s >= len(segs):
                    continue
                dd = segs[s]
                # interleaved within bank: column K*s0 + k*nb + sl
                cols = K * s0 + np.arange(K) * nb + sl
                fta[:, cols] = ft[dd]
                fbias[0, cols] = bias_ns[dd]
                masks[:, s] = (rows == dd).astype(np.float32)
        in_maps.append(
            {
                "eps": np.ascontiguousarray(eps_sorted[c * RPC : (c + 1) * RPC]),
                "fta": fta,
                "fbias": fbias,
                "masks": masks,
                "bsh": bias_sh.reshape(1, NTOP),
            }
        )
    return nseg, in_maps, perm


def _finish(results, perm):
    out_sorted = np.concatenate([results[c]["out"] for c in range(NC)], axis=0)
    out = np.empty((B, N), np.float32)
    out[perm] = out_sorted
    return out


def get_program(nseg):
    prog = _PROG_CACHE.get(nseg)
    if prog is None:
        prog = _build_program(nseg)
        _PROG_CACHE[nseg] = prog
    return prog


def kernel(epsilon, d, L_emb, S_emb, bias_nonshared, bias_shared):
    from concourse.bass_utils import run_bass_kernel_spmd

    nseg, in_maps, perm = _prepare(
        epsilon, d, L_emb, S_emb, bias_nonshared, bias_shared
    )
    prog = get_program(nseg)
    res = run_bass_kernel_spmd(prog, in_maps, list(range(NC))).results
    return _finish(res, perm)
